# revision 33
# baseline (speedup 1.0000x reference)
"""Trainium2 Bass kernel for GQA multi-head attention (nn_MultiHeadAttention).

Problem (hardcoded): B=2, S=2048, DIM=2048, H=32 q-heads, KVH=8 kv-heads,
HD=64, rotate-half RoPE theta=10000, causal, out-proj + bias. All fp32 I/O.

Sharding over 8 NeuronCores (SPMD, one program):
  core c -> batch b=c//4, head-group g=c%4 (q heads 8g..8g+7 = kv heads 2g,2g+1,
  keeping each kv head's 4 q heads together). Each core computes qkv projection
  for its head group (x^T resident in transposed layout), RoPE, causal
  attention with the softmax denominator folded into the AV matmul via an
  appended ones-column on V, and a partial out-projection over its 512 head
  dims. The 4 cores of a batch then ReduceScatter (bf16) the partial
  projections in 4 sequence chunks; each core returns 4x128 rows of the final
  output, which the host concatenates.

Numerics: all matmuls in bf16 with fp32 PSUM accumulation (fp32 matmul is 4x
slower on the PE array); exp on ScalarE in fp32 from PSUM with the 1/sqrt(HD)
scale folded into the activation's free affine; no max-subtraction (scores are
O(5) for these inputs; fp32 exp is exact to ~2ulp and cannot overflow).
"""
import numpy as np
import ml_dtypes

import concourse.bass as bass
import concourse.bacc as bacc
import concourse.tile as tile
import concourse.mybir as mybir
from concourse.bass_utils import run_bass_kernel_spmd

BF16 = mybir.dt.bfloat16
F32 = mybir.dt.float32
AF = mybir.ActivationFunctionType

DIM, H, KVH, HD, B, S = 2048, 32, 8, 64, 2, 2048
NCORES = 8
SCALE = float(1.0 / np.sqrt(HD))
KT = DIM // 128          # 16 contraction tiles
NQC = 4                  # 512-wide sequence chunks
THETA = 10000.0

# sim-verified fast paths (flip off if hardware disagrees)
INPLACE_MASK = True      # in-place mask multiply

_CACHED_NC = None


def _pin_act_tables():
    """Point walrus at a table root containing only natural_log_exp_and_others.

    The kernel's ScalarE functions (Exp, Ln, Copy) all live in that one set,
    but walrus's per-function set choice otherwise thrashes between
    exp_and_others and natural_log (65 ACT_TABLE_LOADs = 83us measured).
    """
    import os
    import tempfile
    import json as _json

    if os.environ.get("BASS_ACT_ROOT_JSON_PATH"):
        return
    import neuronxcc

    src_dir = os.path.join(os.path.dirname(neuronxcc.__file__),
                           "pwp", "pwp_bin_trainium")
    src_json = os.path.join(src_dir, "act_info.json")
    if not os.path.exists(src_json):
        return
    with open(src_json) as f:
        info = _json.load(f)
    keep = [s for s in info["act_func_sets"]
            if s.get("name") == "natural_log_exp_and_others"]
    if not keep:
        return
    info["act_func_sets"] = keep
    dst = tempfile.mkdtemp(prefix="act_pinned_")
    for fn in os.listdir(src_dir):
        if fn != "act_info.json":
            os.symlink(os.path.join(src_dir, fn), os.path.join(dst, fn))
    with open(os.path.join(dst, "act_info.json"), "w") as f:
        _json.dump(info, f)
    os.environ["BASS_ACT_ROOT_JSON_PATH"] = os.path.join(dst, "act_info.json")

    # bacc's pre-placed LoadActFuncSet ids index the same act_info.json that
    # walrus sees; pin its view to the single kept set so ids line up.
    import concourse.hw_specs as hw_specs
    orig = hw_specs.get_activation_tables

    def pinned(arch):
        t = orig(arch)
        return {"natural_log_exp_and_others": t["natural_log_exp_and_others"]}

    hw_specs.get_activation_tables = pinned
    bacc.get_activation_tables = pinned


def build_nc():
    """Build (and cache) the single SPMD Bass program."""
    global _CACHED_NC
    if _CACHED_NC is not None:
        return _CACHED_NC

    _pin_act_tables()
    nc = bacc.Bacc("TRN2", target_bir_lowering=False, debug=False,
                   num_devices=NCORES)

    xt_d = nc.dram_tensor("xt", [DIM, S], F32, kind="ExternalInput")
    wq_d = nc.dram_tensor("wq", [DIM, 512], F32, kind="ExternalInput")
    wk_d = nc.dram_tensor("wk", [DIM, 128], F32, kind="ExternalInput")
    wv_d = nc.dram_tensor("wv", [DIM, 128], F32, kind="ExternalInput")
    wp_d = nc.dram_tensor("wp", [512, DIM], F32, kind="ExternalInput")
    bias_d = nc.dram_tensor("bias", [1, DIM], F32, kind="ExternalInput")
    cos_d = nc.dram_tensor("cost", [128, S], F32, kind="ExternalInput")
    sin_d = nc.dram_tensor("sint", [128, S], F32, kind="ExternalInput")
    r2t_d = nc.dram_tensor("r2t", [128, 128], BF16, kind="ExternalInput")
    mask_d = nc.dram_tensor("maskt", [128, 2048], BF16, kind="ExternalInput")
    y_d = nc.dram_tensor("y", [512, DIM], BF16, kind="ExternalOutput")

    groups = [[0, 1, 2, 3], [4, 5, 6, 7]]

    with tile.TileContext(nc) as tc:
        with (
            tc.tile_pool(name="sb", bufs=1) as sb,
            tc.tile_pool(name="ps", bufs=1, space="PSUM") as ps,
            tc.tile_pool(name="dr", bufs=1, space="DRAM") as dr,
        ):
            # ---- constants / persistent tiles ----
            ones64 = sb.tile([1, 64], BF16, tag="c0", bufs=1)
            nc.vector.memset(ones64[:], 1.0)
            # 0.25 folds the bias/4 scaling (4-way ReduceScatter sums bias 4x)
            ones128 = sb.tile([1, 128], F32, tag="c1", bufs=1)
            nc.vector.memset(ones128[:], 0.25)
            cos_sb = sb.tile([128, S], F32, tag="cos", bufs=1)
            nc.sync.dma_start(cos_sb[:], cos_d[:])
            sin_sb = sb.tile([128, S], F32, tag="sin", bufs=1)
            nc.sync.dma_start(sin_sb[:], sin_d[:])
            r2t_sb = sb.tile([128, 128], BF16, tag="r2t", bufs=1)
            nc.sync.dma_start(r2t_sb[:], r2t_d[:])
            mask_sb = sb.tile([128, 2048], BF16, tag="mask", bufs=1)
            nc.sync.dma_start(mask_sb[:], mask_d[:])

            # bias/4 broadcast to [128, DIM] (the 4-way ReduceScatter sums the
            # bias 4 times, hence the 0.25)
            biasr = sb.tile([1, DIM], F32, tag="biasr", bufs=1)
            nc.sync.dma_start(biasr[:], bias_d[:])
            biasb = sb.tile([128, DIM], F32, tag="biasb", bufs=1)
            for dc in range(4):
                pb = ps.tile([128, 512], F32, tag="av", bufs=2, name="pb")
                nc.tensor.matmul(pb[:], ones128[:], biasr[:, 512 * dc:512 * (dc + 1)],
                                 start=True, stop=True)
                nc.scalar.copy(biasb[:, 512 * dc:512 * (dc + 1)], pb[:])

            # v with ones column: [128 s, 16 stile x 2 kvh x 65] bf16
            vaug = sb.tile([128, 16 * 2 * 65], BF16, tag="vaug", bufs=1)
            va = vaug[:].rearrange("p (t h c) -> p t h c", t=16, h=2, c=65)
            nc.vector.memset(va[:, :, :, 64], 1.0)

            ropedq = [sb.tile([128, S], BF16, tag="ropedq", bufs=4, name=f"rq{i}")
                      for i in range(4)]
            # kv head l duplicated into both 64-row halves so QK matmul operand
            # base partitions match for q heads in either half
            kdup = [sb.tile([128, S], BF16, tag="kdup", bufs=2, name=f"kd{i}")
                    for i in range(2)]
            outt = [sb.tile([128, S], BF16, tag="outt", bufs=4, name=f"ot{i}")
                    for i in range(4)]

            # ---- weights: load fp32, cast to bf16 on DVE ----
            def load_cast(dram_ap, cols, tag, bufs, n_tiles):
                tiles = []
                for kt in range(n_tiles):
                    st = sb.tile([128, cols], F32, tag="wstage", bufs=4, name="wst")
                    nc.sync.dma_start(st[:], dram_ap[128 * kt:128 * (kt + 1), :cols])
                    t = sb.tile([128, cols], BF16, tag=tag, bufs=bufs, name=tag)
                    nc.vector.tensor_copy(t[:], st[:])
                    tiles.append(t)
                return tiles

            # interleave wk/wv/wq per k-tile so the first qkv accumulations
            # can start as soon as the first tiles land
            wq_sb, wk_sb, wv_sb = [], [], []
            for kt in range(KT):
                wk_sb += load_cast(wk_d[128 * kt:, :], 128, "wk", KT, 1)
                wv_sb += load_cast(wv_d[128 * kt:, :], 128, "wv", KT, 1)
                wq_sb += load_cast(wq_d[128 * kt:, :], 512, "wq", KT, 1)
            wp_sb = [sb.tile([128, DIM], BF16, tag="wp", bufs=4, name="wp")
                     for hk in range(4)]

            def load_wp():
                # emitted at the start of the first attention phase: DMA+cast
                # overlap attention, ready before proj(qc=0)
                for hk in range(4):
                    for dc in range(4):
                        st = sb.tile([128, 512], F32, tag="xstage", bufs=4,
                                     name="wpst")
                        nc.sync.dma_start(
                            st[:],
                            wp_d[128 * hk:128 * (hk + 1), 512 * dc:512 * (dc + 1)])
                        nc.vector.tensor_copy(
                            wp_sb[hk][:, 512 * dc:512 * (dc + 1)], st[:])

            yp_dr = [dr.tile([512, DIM], BF16, tag=f"yp{qc}", bufs=1, name=f"yp{qc}")
                     for qc in range(NQC)]
            yrs_dr = [dr.tile([128, DIM], BF16, tag=f"yrs{qc}", bufs=1,
                              name=f"yrs{qc}")
                      for qc in range(NQC)]
            # contiguous half-width buffers for the final chunk's split RS
            yp3h = [dr.tile([512, DIM // 2], BF16, tag=f"yp3h{i}", bufs=1,
                            name=f"yp3h{i}") for i in range(2)]
            yrs3h = [dr.tile([128, DIM // 2], BF16, tag=f"yrs3h{i}", bufs=1,
                             name=f"yrs3h{i}") for i in range(2)]

            def rope_chunk(psum_q, ch, dest, k_mode=False):
                """dest[:, 512ch:+512] = psum_q*cos + (R2@bf16(psum_q))*sin.

                k_mode: dest is the kdup pair; head 0 -> kdup[0] rows 0:64,
                head 1 -> kdup[1] rows 64:128, other halves filled by DMA."""
                sl = slice(512 * ch, 512 * (ch + 1))
                q_sb = sb.tile([128, 512], BF16, tag="qsb", bufs=2, name="qsb")
                nc.scalar.copy(q_sb[:], psum_q[:])
                prot = ps.tile([128, 512], F32, tag="mm", bufs=2, name="prot")
                nc.tensor.matmul(prot[:], r2t_sb[:], q_sb[:], start=True, stop=True)
                e1 = sb.tile([128, 512], F32, tag="e1", bufs=2, name="e1")
                nc.vector.tensor_mul(e1[:], psum_q[:], cos_sb[:, sl])
                e2 = sb.tile([128, 512], F32, tag="e2", bufs=2, name="e2")
                nc.vector.tensor_mul(e2[:], prot[:], sin_sb[:, sl])
                if not k_mode:
                    nc.vector.tensor_add(dest[:, sl], e1[:], e2[:])
                else:
                    kd0, kd1 = dest
                    nc.vector.tensor_add(kd0[0:64, sl], e1[0:64, :], e2[0:64, :])
                    nc.vector.tensor_add(kd1[64:128, sl], e1[64:128, :],
                                         e2[64:128, :])
                    nc.sync.dma_start(kd0[64:128, sl], kd0[0:64, sl])
                    nc.sync.dma_start(kd1[0:64, sl], kd1[64:128, sl])

            # ================= software-pipelined main loop ===================
            # Emission order interleaves three streams so every engine stays
            # dense: attention heads for chunk qc, next chunk's qkv projection
            # (PE filler while ACT drains exps), and the previous chunk's
            # out-projection + ReduceScatter.

            def b_phase_pieces(ch):
                """Next-chunk qkv work split into 8 pieces (one per head)."""
                sl = slice(512 * ch, 512 * (ch + 1))
                xbf = []

                def x_piece(i0):
                    def go():
                        for kt in range(i0, i0 + 4):
                            st = sb.tile([128, 512], F32, tag="xstage", bufs=4,
                                         name="xst")
                            nc.sync.dma_start(
                                st[:], xt_d[128 * kt:128 * (kt + 1), sl])
                            xb = sb.tile([128, 512], BF16, tag="xbf", bufs=20,
                                         name="xbf")
                            nc.vector.tensor_copy(xb[:], st[:])
                            xbf.append(xb)
                    return go

                def k_piece():
                    pk = ps.tile([128, 512], F32, tag="mm", bufs=2, name="pk")
                    for kt in range(KT):
                        nc.tensor.matmul(pk[:], wk_sb[kt][:], xbf[kt][:],
                                         start=(kt == 0), stop=(kt == KT - 1))
                    rope_chunk(pk, ch, kdup, k_mode=True)

                def v_piece():
                    for p in range(4):
                        st_idx = 4 * ch + p
                        pv = ps.tile([128, 128], F32, tag="mm", bufs=2, name="pv")
                        for kt in range(KT):
                            nc.tensor.matmul(
                                pv[:], xbf[kt][:, 128 * p:128 * (p + 1)],
                                wv_sb[kt][:],
                                start=(kt == 0), stop=(kt == KT - 1))
                        pvv = pv[:].rearrange("p (h c) -> p h c", h=2)
                        nc.vector.tensor_copy(va[:, st_idx, :, 0:64], pvv[:])

                def q_piece(qts):
                    def go():
                        for qt in qts:
                            pq = ps.tile([128, 512], F32, tag="mm", bufs=2,
                                         name="pq")
                            for kt in range(KT):
                                nc.tensor.matmul(
                                    pq[:], wq_sb[kt][:, 128 * qt:128 * (qt + 1)],
                                    xbf[kt][:],
                                    start=(kt == 0), stop=(kt == KT - 1))
                            rope_chunk(pq, ch, ropedq[qt])
                    return go

                return [x_piece(0), x_piece(4), x_piece(8), x_piece(12),
                        k_piece, v_piece, q_piece([0, 1]), q_piece([2, 3])]

            def attention_head(qc, h):
                lkv = h // 4
                qrows = slice(64 * (h % 2), 64 * (h % 2) + 64)
                krows = qrows           # kdup holds the kv head in both halves
                ktile = kdup[lkv]
                qtile = ropedq[h // 2]
                po = ps.tile([65, 512], F32, tag="av", bufs=2, name="po")
                n_grp = 2 * (qc + 1)            # groups of 2 kv-tiles
                for grp in range(n_grp):
                    pscr = ps.tile([128, 1024], F32, tag="scores", bufs=2,
                                   name="pscr")
                    for j in range(2):
                        tkv = 2 * grp + j
                        nc.tensor.matmul(
                            pscr[:, 512 * j:512 * (j + 1)],
                            ktile[krows, 128 * tkv:128 * (tkv + 1)],
                            qtile[qrows, 512 * qc:512 * (qc + 1)],
                            start=True, stop=True)
                    expt = sb.tile([128, 1024], BF16, tag="expt", bufs=6,
                                   name="expt")
                    nc.scalar.activation(expt[:], pscr[:], AF.Exp, scale=SCALE)
                    for j in range(2):
                        tkv = 2 * grp + j
                        p = tkv - 4 * qc
                        if p >= 0:              # diagonal block: causal mask
                            w = 128 * (p + 1)
                            reg = expt[:, 512 * j:512 * j + w]
                            msk = mask_sb[:, 512 * p:512 * p + w]
                            nc.vector.tensor_mul(reg[:], reg[:], msk[:])
                    for j in range(2):
                        tkv = 2 * grp + j
                        nc.tensor.matmul(
                            po[:], va[:, tkv, lkv, 0:65],
                            expt[:, 512 * j:512 * (j + 1)],
                            start=(grp == 0 and j == 0),
                            stop=(grp == n_grp - 1 and j == 1))
                # normalize: outT = po[0:64] * (1/po[64]); 1/Z = exp(-ln Z) on
                # ScalarE (same ACT table set as the attention exp; DVE
                # reciprocal() is lane-starved on [1, 512])
                lnz = sb.tile([1, 512], F32, tag="lnz", bufs=3, name="lnz")
                nc.scalar.activation(lnz[:], po[64:65, :], AF.Ln)
                recip = sb.tile([1, 512], BF16, tag="recip", bufs=3,
                                name="recip")
                nc.scalar.activation(recip[:], lnz[:], AF.Exp, scale=-1.0)
                pr = ps.tile([64, 512], F32, tag="av", bufs=2, name="pr")
                nc.tensor.matmul(pr[:], ones64[:], recip[:], start=True, stop=True)
                rbc = sb.tile([64, 512], F32, tag="rbc", bufs=2, name="rbc")
                nc.vector.tensor_copy(rbc[:], pr[:])
                dst = outt[h // 2][qrows, 512 * qc:512 * (qc + 1)]
                nc.vector.tensor_mul(dst[:], po[0:64, :], rbc[:])

            def proj_stile(qc, p, dcs=(0, 1, 2, 3), dst=None, col_base=0):
                st_idx = 4 * qc + p
                if dst is None:
                    dst = yp_dr[qc]
                for dc in dcs:
                    py = ps.tile([128, 512], F32, tag="av", bufs=2, name="py")
                    for hk in range(4):
                        nc.tensor.matmul(
                            py[:], outt[hk][:, 128 * st_idx:128 * (st_idx + 1)],
                            wp_sb[hk][:, 512 * dc:512 * (dc + 1)],
                            start=(hk == 0), stop=(hk == 3))
                    ysb = sb.tile([128, 512], BF16, tag="ysb", bufs=3, name="ysb")
                    nc.vector.tensor_add(ysb[:], py[:],
                                         biasb[:, 512 * dc:512 * (dc + 1)])
                    c0 = 512 * dc - col_base
                    nc.sync.dma_start(
                        dst[128 * p:128 * (p + 1), c0:c0 + 512], ysb[:])

            def rs_and_out(qc):
                nc.gpsimd.collective_compute(
                    "ReduceScatter", mybir.AluOpType.add, replica_groups=groups,
                    ins=[yp_dr[qc][:]], outs=[yrs_dr[qc][:]])

            # chunk 0 qkv up front (nothing to hide it behind)
            for piece in b_phase_pieces(0):
                piece()
            for ch in range(NQC):
                if ch == 0:
                    load_wp()
                nextb = b_phase_pieces(ch + 1) if ch < NQC - 1 else None
                for h in range(8):
                    attention_head(ch, h)
                    if nextb is not None:
                        nextb[h]()
                    if ch >= 1 and h < 4:
                        proj_stile(ch - 1, h)
                        if h == 3:
                            rs_and_out(ch - 1)
                if ch == NQC - 1:
                    # final chunk: project in two column halves with a split
                    # ReduceScatter so the second half overlaps the first RS
                    for half in range(2):
                        dcs = (0, 1) if half == 0 else (2, 3)
                        for p in range(4):
                            proj_stile(ch, p, dcs=dcs, dst=yp3h[half],
                                       col_base=1024 * half)
                        nc.gpsimd.collective_compute(
                            "ReduceScatter", mybir.AluOpType.add,
                            replica_groups=groups,
                            ins=[yp3h[half][:]], outs=[yrs3h[half][:]])

            # output copies last: every RS has fired; nothing queues behind them
            for qc in range(NQC - 1):
                nc.sync.dma_start(y_d[128 * qc:128 * (qc + 1), :], yrs_dr[qc][:])
            for half in range(2):
                nc.sync.dma_start(
                    y_d[128 * (NQC - 1):128 * NQC,
                        1024 * half:1024 * (half + 1)], yrs3h[half][:])


    nc.compile()
    _CACHED_NC = nc
    return nc


def _consts():
    half = HD // 2
    inv_freq = 1.0 / (THETA ** (np.arange(half, dtype=np.float32) * 2.0 / HD))
    ang = np.arange(S, dtype=np.float32)[:, None] * inv_freq      # [S, 32]
    cos = np.cos(ang).T.astype(np.float32)                        # [32, S]
    sin = np.sin(ang).T.astype(np.float32)
    cos64 = np.concatenate([cos, cos], 0)
    sin64 = np.concatenate([sin, sin], 0)
    cosT = np.concatenate([cos64, cos64], 0)                      # [128, S]
    sinT = np.concatenate([sin64, sin64], 0)

    M = np.zeros((HD, HD), np.float32)
    for i in range(half):
        M[i, i + half] = -1.0
        M[i + half, i] = 1.0
    M2 = np.zeros((128, 128), np.float32)
    M2[:64, :64] = M
    M2[64:, 64:] = M
    r2t = M2.T.astype(ml_dtypes.bfloat16)

    masks = np.zeros((128, 2048), np.float32)
    q_idx = np.arange(512)[None, :]
    for p in range(4):
        kv_idx = np.arange(128)[:, None] + 128 * p
        masks[:, 512 * p:512 * (p + 1)] = (q_idx >= kv_idx)
    maskt = masks.astype(ml_dtypes.bfloat16)
    return cosT, sinT, r2t, maskt


def _in_maps(x, w_qkv, w_proj, b_proj):
    cosT, sinT, r2t, maskt = _consts()
    f32 = np.float32
    maps = []
    for c in range(NCORES):
        b, g = c // 4, c % 4
        maps.append({
            "xt": np.ascontiguousarray(x[b].T, dtype=f32),
            "wq": np.ascontiguousarray(w_qkv[:, 512 * g:512 * (g + 1)], dtype=f32),
            "wk": np.ascontiguousarray(
                w_qkv[:, 2048 + 128 * g:2048 + 128 * (g + 1)], dtype=f32),
            "wv": np.ascontiguousarray(
                w_qkv[:, 2560 + 128 * g:2560 + 128 * (g + 1)], dtype=f32),
            "wp": np.ascontiguousarray(w_proj[512 * g:512 * (g + 1), :], dtype=f32),
            "bias": np.ascontiguousarray(b_proj[None, :], dtype=f32),
            "cost": cosT, "sint": sinT, "r2t": r2t, "maskt": maskt,
        })
    return maps


def _assemble(results):
    out = np.zeros((B, S, DIM), np.float32)
    for c in range(NCORES):
        b, j = c // 4, c % 4
        y = results[c]["y"]                    # [512, DIM]
        for qc in range(NQC):
            rows = slice(512 * qc + 128 * j, 512 * qc + 128 * (j + 1))
            out[b, rows, :] = y[128 * qc:128 * (qc + 1), :]
    return out


def run(x, w_qkv, w_proj, b_proj, trace=False):
    nc = build_nc()
    res = run_bass_kernel_spmd(nc, _in_maps(x, w_qkv, w_proj, b_proj),
                               core_ids=list(range(NCORES)), trace=trace)
    return _assemble(res.results), res


def kernel(x, w_qkv, w_proj, b_proj):
    x = np.asarray(x)
    w_qkv = np.asarray(w_qkv)
    w_proj = np.asarray(w_proj)
    b_proj = np.asarray(b_proj)
    out, _ = run(x, w_qkv, w_proj, b_proj, trace=False)
    return out
